# revision 5
# baseline (speedup 1.0000x reference)
"""Multi-head causal attention (B=2, S=2048, D=1024, H=16) on 8 trn2 NeuronCores.

Strategy (tensor-parallel over heads, per the sharding hint):
  - Each core owns 2 heads (128 of 1024 hidden dims): W_q/W_k/W_v column-parallel.
  - Activations kept transposed ([dim, token]) end to end so every matmul
    contracts on the partition axis with zero on-device transposes of x.
  - scores^T = K^T.T @ Q^T per 128-key-chunk x 512-query-tile, two heads packed
    into disjoint PE row-groups (contraction is only dk=64) - they run
    concurrently in the array.
  - softmax without max-subtraction (scores are O(1)); rowsum folded into the
    PV matmul via an augmented V [keys, 64+1] whose last column is ones.
  - exp only on the causal part of diagonal chunks; the rest of the P tile is
    zeroed, and only the 128-wide diagonal strip is tri-masked.
  - low-latency softmax normalization: reciprocal of the rowsum row straight
    out of PSUM, broadcast across 64 partitions by a rank-1 PE matmul
    (ones[1,64]^T @ r[1,512]), then one fused DVE multiply.  No SWDGE hops.
  - a dummy warmup AllToAll fires at t~=2us to absorb the one-time ~15us
    CC-core launch latency before any real collective needs it.
  - ctx re-sharded token-parallel: one AllToAll for batch 0 (fired mid-kernel,
    its out-projection overlaps batch-1 attention) and TWO half-batch
    AllToAlls for batch 1 (heavy-first q-tile order j=3,2,1,0) so the final
    collective only carries the two cheapest q-tiles and half the batch-1
    out-projection overlaps the collective wait.
  - DMA trigger discipline: a gated DMA trigger is an *instruction* on its
    engine's sequencer and blocks everything behind it there.  The Sync ring
    is ordered by guaranteed completion time (x tiles first - all 8 resident -
    then a2a_in writes in attention order, with collective-gated ctx loads
    placed only where their gate provably clears first).  Startup constants
    ride the Scalar ring; W_o/b_o the GpSimd SWDGE ring.
  - bf16 matmul inputs everywhere; PSUM accumulation stays fp32.

kernel(**inputs) takes the full unsharded inputs and returns the full output.
"""

import numpy as np
import ml_dtypes

import concourse.bass as bass
import concourse.mybir as mybir
import concourse.tile as tile
from concourse import bacc
from concourse.bass import ts
from concourse.bass_utils import run_bass_kernel_spmd
from concourse.tile_rust import add_dep_helper

B, S, D = 2, 2048, 1024
H, DK = 16, 64
NCORE = 8
T = B * S          # 4096 tokens
TT = 512           # token tile (projections, q-tiles)
NT = T // TT       # 8
KC = 128           # key chunk
NJ = S // TT       # 4 q-tiles per batch
GG = 256           # batch-0 a2a token group (per dst core)
PG = 128           # batch-1 a2a token group (per dst core, per pair)
SCALE = 1.0 / np.sqrt(DK)

f32 = mybir.dt.float32
bf16 = mybir.dt.bfloat16
EXP = mybir.ActivationFunctionType.Exp
MULT = mybir.AluOpType.mult
npbf = ml_dtypes.bfloat16


def build_program():
    nc = bacc.Bacc("TRN2", target_bir_lowering=False, debug=False,
                   num_devices=NCORE)

    xT_d = nc.dram_tensor("xT", [NT, 128, 8, TT], bf16, kind="ExternalInput").ap()
    wT_d = nc.dram_tensor("wT", [128, 8, 3, 128], bf16, kind="ExternalInput").ap()
    woT_d = nc.dram_tensor("woT", [128, 8, 1024], bf16, kind="ExternalInput").ap()
    bqkv_d = nc.dram_tensor("bqkv", [128, 3], f32, kind="ExternalInput").ap()
    bo_d = nc.dram_tensor("bo", [128, 1024], f32, kind="ExternalInput").ap()
    trimask_d = nc.dram_tensor("trimask", [128, 128], bf16, kind="ExternalInput").ap()
    ident_d = nc.dram_tensor("ident", [128, 128], bf16, kind="ExternalInput").ap()
    # outT_d[0, th, t, od] = batch-0 token 256*core + 128*th + t
    # outT_d[1, p, t, od]  = batch-1 token 1024*(1-p) + 128*core + t
    outT_d = nc.dram_tensor("outT", [B, 2, 128, 1024], f32, kind="ExternalOutput").ap()

    with tile.TileContext(nc) as tc:
        with (
            tc.tile_pool(name="const", bufs=1) as constp,
            tc.tile_pool(name="wostream", bufs=1) as wop,
            tc.tile_pool(name="xstream", bufs=NT) as xp,
            tc.tile_pool(name="qkv", bufs=NT) as qkvp,
            tc.tile_pool(name="vaug", bufs=NJ) as vaugp,
            tc.tile_pool(name="ptile", bufs=4) as pp,
            tc.tile_pool(name="cxn", bufs=4) as cxnp,
            tc.tile_pool(name="cxhold", bufs=4) as cxp,
            tc.tile_pool(name="outsb", bufs=8) as outp,
            tc.tile_pool(name="ps_s", bufs=2, space="PSUM") as ps_s,
            tc.tile_pool(name="ps_ctx", bufs=1, space="PSUM") as ps_ctx,
            tc.tile_pool(name="ps_misc", bufs=2, space="PSUM") as ps_misc,
            tc.tile_pool(name="dram", bufs=1, space="DRAM") as dramp,
        ):
            # ---- constants on the Scalar HWDGE ring (parallel with x) ----
            wT = constp.tile([128, 8, 3, 128], bf16, tag="wT")
            nc.scalar.dma_start(wT[:], wT_d)
            bqkv = constp.tile([128, 3], f32, tag="bqkv")
            nc.scalar.dma_start(bqkv[:], bqkv_d)
            ident = constp.tile([128, 128], bf16, tag="ident")
            nc.scalar.dma_start(ident[:], ident_d)
            trimask = constp.tile([128, 128], bf16, tag="trimask")
            nc.scalar.dma_start(trimask[:], trimask_d)
            ones64 = constp.tile([1, DK], bf16, tag="ones64")
            nc.vector.memset(ones64[:], 1.0)

            qkv_t = [[None] * NT for _ in range(3)]   # [j][t] -> [128, TT]
            vaug_t = [[[None] * NJ for _ in range(2)] for _ in range(B)]
            x_t = [None] * NT

            # batch-0: one a2a (dst core c takes tokens [256c, 256c+256));
            # batch-1: one a2a per pair of q-tiles, p=0 -> {3,2} (tokens
            # [1024,2048)), p=1 -> {1,0}; dst c takes 128 tokens at offset
            # 128c within the pair's range.
            a2a_in = {0: dramp.tile([NCORE, 128, GG], bf16, name="a2a_in0"),
                      (1, 0): dramp.tile([NCORE, 128, PG], bf16, name="a2a_in10"),
                      (1, 1): dramp.tile([NCORE, 128, PG], bf16, name="a2a_in11")}
            a2a_out = {0: dramp.tile([NCORE, 128, GG], bf16, name="a2a_out0"),
                       (1, 0): dramp.tile([NCORE, 128, PG], bf16, name="a2a_out10"),
                       (1, 1): dramp.tile([NCORE, 128, PG], bf16, name="a2a_out11")}
            warm_in = dramp.tile([NCORE, 4, 32], f32, name="warm_in")
            warm_out = dramp.tile([NCORE, 4, 32], f32, name="warm_out")

            def load_x(t):
                xt = xp.tile([128, 8, TT], bf16, tag="xt", name=f"xt{t}")
                if t == 0:
                    nc.sync.dma_start(xt[:, 0:2, :], xT_d[t, :, 0:2, :])
                    nc.sync.dma_start(xt[:, 2:8, :], xT_d[t, :, 2:8, :])
                else:
                    nc.sync.dma_start(xt[:], xT_d[t])
                x_t[t] = xt

            def proj_mms(t):
                xt = x_t[t]
                for j in range(3):
                    ps = ps_misc.tile([128, TT], f32, tag="mm")
                    for o in range(8):
                        nc.tensor.matmul(ps[:], wT[:, o, j, :], xt[:, o, :],
                                         start=(o == 0), stop=(o == 7))
                    qt = qkvp.tile([128, TT], bf16, tag=f"qkv{j}",
                                   name=f"qkv{j}_{t}")
                    nc.vector.tensor_scalar_add(qt[:], ps[:], bqkv[:, j:j + 1])
                    qkv_t[j][t] = qt

            def vtrans_tile(t):
                b, tl = t // NJ, t % NJ
                va = [vaugp.tile([128, NJ, DK + 1], bf16, tag=f"va{b}{h}",
                                 name=f"va{b}{h}_{tl}") for h in range(2)]
                for h in range(2):
                    nc.vector.memset(va[h][:, :, DK:DK + 1], 1.0)
                    vaug_t[b][h][tl] = va[h]
                for kt in range(NJ):
                    ps_t = ps_misc.tile([128, TT], bf16, tag="mm")
                    nc.tensor.transpose(ps_t[:, 0:128],
                                        qkv_t[2][t][:, kt * KC:(kt + 1) * KC],
                                        ident[:])
                    for h in range(2):
                        nc.vector.tensor_copy(va[h][:, kt, 0:DK],
                                              ps_t[:, DK * h:DK * h + DK])

            def attention_qtile(b, j, pre=None):
                """Emit one q-tile's attention.  Returns a closure that emits
                the softmax normalization + a2a_in shipping; pass it as `pre`
                to the NEXT q-tile (it is emitted after that tile's second
                score chunk so the PE has runway while the DVE computes the
                reciprocal), or call it directly."""
                nk = 4 * (j + 1)
                pc = [ps_ctx.tile([DK + 1, TT], f32, tag=f"c{h}", name=f"pc{h}")
                      for h in range(2)]

                def emit_pv(p_tile, m):
                    for h in range(2):
                        nc.tensor.matmul(
                            pc[h][:], vaug_t[b][h][m // 4][:, m % 4, :],
                            p_tile[:, TT * h:TT * (h + 1)],
                            start=(m == 0), stop=(m == nk - 1),
                            skip_group_check=True)

                qt = qkv_t[0][b * NJ + j]
                pending = []
                for m in range(nk):
                    kt_tile = qkv_t[1][b * NJ + m // 4]
                    ko = (m % 4) * KC
                    ps = ps_s.tile([128, 2 * TT], f32, tag="s")
                    nc.tensor.matmul(ps[:, 0:TT], kt_tile[0:DK, ko:ko + KC],
                                     qt[0:DK, :],
                                     start=True, stop=True, tile_position=(0, 0))
                    nc.tensor.matmul(ps[:, TT:], kt_tile[DK:128, ko:ko + KC],
                                     qt[DK:128, :],
                                     start=True, stop=True, tile_position=(64, 0))
                    p = pp.tile([128, 2 * TT], bf16, tag="p")
                    r = m - 4 * j
                    if r >= 0:
                        if r > 0:
                            nc.vector.memset(
                                p[:].rearrange("k (h q) -> k h q", h=2)[:, :, 0:KC * r],
                                0.0)
                        nc.scalar.activation(
                            p[:].rearrange("k (h q) -> k h q", h=2)[:, :, KC * r:],
                            ps[:].rearrange("k (h q) -> k h q", h=2)[:, :, KC * r:],
                            EXP, scale=float(SCALE))
                        nc.vector.tensor_tensor(
                            p[:].rearrange("k (h q) -> k h q", h=2)[:, :, KC * r:KC * (r + 1)],
                            p[:].rearrange("k (h q) -> k h q", h=2)[:, :, KC * r:KC * (r + 1)],
                            trimask[:, None, :].to_broadcast([128, 2, 128]), MULT)
                    else:
                        nc.scalar.activation(p[:], ps[:], EXP, scale=float(SCALE))
                    pending.append((p, m))
                    if m == 1 and pre is not None:
                        pre()
                    if len(pending) > 2:   # depth-2: PE never waits on a fresh exp
                        emit_pv(*pending.pop(0))
                for pm in pending:
                    emit_pv(*pm)

                def norm_and_ship():
                    for h in range(2):
                        rr = cxnp.tile([1, TT], bf16, tag="rr")
                        with nc.allow_low_precision(reason="softmax denominator"):
                            nc.vector.reciprocal(rr[:], pc[h][DK:DK + 1, :])
                        cx = cxp.tile([DK, TT], f32, tag="cx")
                        nc.vector.tensor_copy(cx[:], pc[h][0:DK, :])
                        bc = ps_misc.tile([DK, TT], f32, tag="mm")
                        nc.tensor.matmul(bc[:], ones64[:], rr[:],
                                         start=True, stop=True)
                        cxn = cxnp.tile([DK, TT], bf16, tag="cxn")
                        nc.vector.tensor_tensor(cxn[:], cx[:], bc[:], MULT)
                        if b == 0:
                            for g in range(2):   # 256-token groups, dst 2j+g
                                nc.sync.dma_start(
                                    a2a_in[0][2 * j + g, DK * h:DK * (h + 1), :],
                                    cxn[:, GG * g:GG * (g + 1)])
                        else:
                            p_pair = (3 - j) // 2
                            dst0 = 4 * (j - (2 - 2 * p_pair))
                            for g in range(4):   # 128-token groups
                                nc.sync.dma_start(
                                    a2a_in[(1, p_pair)][dst0 + g,
                                                        DK * h:DK * (h + 1), :],
                                    cxn[:, PG * g:PG * (g + 1)])
                return norm_and_ship

            def do_a2a(key, in_t, out_t):
                nc.gpsimd.collective_compute(
                    "AllToAll", mybir.AluOpType.bypass,
                    replica_groups=[list(range(NCORE))],
                    ins=[in_t[:].opt()], outs=[out_t[:].opt()])

            ctx_tiles = {}

            def load_ctx(key, width):
                ctx_sb = constp.tile([128, 8, width], bf16, tag=f"ctx{key}",
                                     name=f"ctx{key}")
                for d in range(8):
                    nc.sync.dma_start(ctx_sb[:, d, :], a2a_out[key][d])
                ctx_tiles[key] = ctx_sb

            out_tiles = {}

            def outproj_mms(key, bslot, pslot, toks):
                ctx_sb = ctx_tiles[key]
                for th in range(toks // 128):   # 128-token groups
                    for oh in range(2):         # 512-wide od halves
                        ps = ps_misc.tile([128, TT], f32, tag="mm")
                        for d in range(8):
                            nc.tensor.matmul(
                                ps[:], ctx_sb[:, d, KC * th:KC * (th + 1)],
                                wo_sb[:, d, TT * oh:TT * (oh + 1)],
                                start=(d == 0), stop=(d == 7))
                        ot = outp.tile([128, TT], f32, tag="ot")
                        nc.vector.tensor_tensor(
                            ot[:], ps[:], bo_sb[:, TT * oh:TT * (oh + 1)],
                            mybir.AluOpType.add)
                        out_tiles[(bslot, pslot + th, oh)] = ot

            def out_store(bslot, pslot, ngrp=1):
                for th in range(ngrp):
                    for oh in range(2):
                        nc.sync.dma_start(
                            outT_d[bslot, pslot + th, :, TT * oh:TT * (oh + 1)],
                            out_tiles[(bslot, pslot + th, oh)][:])

            # ---- schedule ----
            load_x(0)
            load_x(1)
            load_x(2)
            load_x(3)
            # warm up the CC core early: the first collective pays ~15us of
            # launch latency; burn it on a dummy while the PE does projections
            do_a2a("warm", warm_in, warm_out)
            wo_sb = wop.tile([128, 8, 1024], bf16, tag="wo")
            nc.gpsimd.dma_start(wo_sb[:], woT_d)
            bo_sb = wop.tile([128, 1024], f32, tag="bobc")
            nc.gpsimd.dma_start(bo_sb[:], bo_d)
            proj_mms(0)
            load_x(4)
            load_x(5)
            load_x(6)
            load_x(7)

            # batch 0, v1-style pipeline, natural order
            norm = None
            for g in range(NJ):
                if g + 1 < NT:
                    proj_mms(g + 1)
                vtrans_tile(g)
                norm = attention_qtile(0, g, pre=norm)
            proj_mms(5)
            norm()                      # norm(0,3): PE runway = proj_mms(5)
            do_a2a(0, a2a_in[0], a2a_out[0])
            vtrans_tile(4)
            proj_mms(6)
            vtrans_tile(5)
            proj_mms(7)
            vtrans_tile(6)
            vtrans_tile(7)

            # batch 1, heavy-first pairs
            norm = attention_qtile(1, 3, pre=None)
            norm = attention_qtile(1, 2, pre=norm)
            load_ctx(0, GG)             # gate a2a(0): cleared long ago
            outproj_mms(0, 0, 0, GG)
            norm = attention_qtile(1, 1, pre=norm)
            do_a2a((1, 0), a2a_in[(1, 0)], a2a_out[(1, 0)])
            norm = attention_qtile(1, 0, pre=norm)
            norm()                      # norm(1,0): straight to the tail
            do_a2a((1, 1), a2a_in[(1, 1)], a2a_out[(1, 1)])
            out_store(0, 0, ngrp=2)
            load_ctx((1, 0), PG)
            outproj_mms((1, 0), 1, 0, PG)
            out_store(1, 0)
            load_ctx((1, 1), PG)
            outproj_mms((1, 1), 1, 1, PG)
            out_store(1, 1)

    nc.compile()
    return nc


def make_in_maps(x, Wq, bq, Wk, bk, Wv, bv, Wo, bo):
    x = np.asarray(x, np.float32)
    xT = np.ascontiguousarray(x.reshape(T, D).T)                  # [D, T]
    # [NT, 128, 8, TT]: xT_t[t, p, o, q] = xT[o*128+p, t*TT+q]
    xT_t = np.ascontiguousarray(
        xT.reshape(8, 128, NT, TT).transpose(2, 1, 0, 3)).astype(npbf)

    woT = np.ascontiguousarray(
        np.asarray(Wo, np.float32).T.reshape(8, 128, 1024)
        .transpose(1, 0, 2)).astype(npbf)
    bo_bc = np.ascontiguousarray(
        np.broadcast_to(np.asarray(bo, np.float32)[None, :], (128, 1024)))

    trimask = (np.arange(128)[:, None] <= np.arange(128)[None, :]).astype(npbf)
    ident = np.eye(128, dtype=npbf)

    in_maps = []
    for c in range(NCORE):
        sl = slice(128 * c, 128 * (c + 1))
        wT_c = np.stack(
            [np.ascontiguousarray(
                np.asarray(W, np.float32)[sl, :].T.reshape(8, 128, 128)
                .transpose(1, 0, 2))
             for W in (Wq, Wk, Wv)], axis=2)                       # [128, 8, 3, 128]
        bqkv_c = np.stack([np.asarray(b_, np.float32)[sl]
                           for b_ in (bq, bk, bv)], axis=1)        # [128, 3]
        in_maps.append({
            "xT": xT_t,
            "wT": np.ascontiguousarray(wT_c).astype(npbf),
            "woT": woT,
            "bqkv": np.ascontiguousarray(bqkv_c),
            "bo": bo_bc,
            "trimask": trimask,
            "ident": ident,
        })
    return in_maps


def assemble_output(results):
    # batch 0: results[c]["outT"][0, th] = tokens 256c + 128*th .. +128
    # batch 1: results[c]["outT"][1, p]  = tokens 1024*(1-p) + 128c .. +128
    out = np.empty((B, S, D), np.float32)
    for c in range(NCORE):
        r = results[c]["outT"]
        out[0, GG * c:GG * (c + 1), :] = r[0].reshape(GG, D)
        for p in range(2):
            base = 1024 * (1 - p) + PG * c
            out[1, base:base + PG, :] = r[1, p]
    return out


_PROGRAM = None


def get_program():
    global _PROGRAM
    if _PROGRAM is None:
        _PROGRAM = build_program()
    return _PROGRAM


def run(in_maps, **kwargs):
    nc = get_program()
    return run_bass_kernel_spmd(nc, in_maps, core_ids=list(range(NCORE)), **kwargs)


def kernel(x, Wq, bq, Wk, bk, Wv, bv, Wo, bo):
    in_maps = make_in_maps(x, Wq, bq, Wk, bk, Wv, bv, Wo, bo)
    res = run(in_maps)
    return assemble_output(res.results)


if __name__ == "__main__":
    rng = np.random.default_rng(0)
    x = rng.standard_normal((B, S, D), dtype=np.float32)
    mk = lambda *s: ((rng.random(s).astype(np.float32)) - 0.5) / 16
    out = kernel(x, mk(D, D), mk(D), mk(D, D), mk(D), mk(D, D), mk(D),
                 mk(D, D), mk(D))
    print(out.shape, out.dtype, np.abs(out).mean())


# revision 6
# speedup vs baseline: 1.2561x; 1.2561x over previous
"""Multi-head causal attention (B=2, S=2048, D=1024, H=16) on 8 trn2 NeuronCores.

Strategy (tensor-parallel over heads, per the sharding hint):
  - Each core owns 2 heads (128 of 1024 hidden dims): W_q/W_k/W_v column-parallel.
  - Activations kept transposed ([dim, token]) end to end so every matmul
    contracts on the partition axis with zero on-device transposes of x.
  - Fully software-pipelined: for each 512-token tile, project Q/K/V,
    transpose V, then run that q-tile's causal attention - the PE never waits
    for a separate projection phase.
  - scores^T = K^T.T @ Q^T per 128-key-chunk x 512-query-tile, two heads packed
    into disjoint PE row-groups (contraction is only dk=64).
  - softmax without max-subtraction (scores are O(1)); rowsum folded into the
    PV matmul via an augmented V [keys, 64+1] whose last column is ones.
  - exp only on the causal part of diagonal chunks; the rest of the P tile is
    zeroed, and only the 128-wide diagonal strip is tri-masked.
  - reciprocals batched into one tiny [128, 8] DVE op per q-tile; row broadcast
    on the otherwise-idle GpSimd engine.
  - a dummy warmup AllToAll fires at t~=2us to absorb the one-time ~15us
    CC-core launch latency before any real collective needs it.
  - ctx re-sharded token-parallel with one AllToAll per batch element; the
    batch-0 AllToAll overlaps batch-1 attention. Out-projection runs with full
    W_o on each core for its 2x256 tokens.
  - startup: constants ride the Scalar HWDGE ring in parallel with the x
    tiles on the Sync ring, and the first x tile is split so the opening
    matmul starts after ~256KB.
  - bf16 matmul inputs everywhere; PSUM accumulation and softmax
    normalization stay fp32.

kernel(**inputs) takes the full unsharded inputs and returns the full output.
"""

import numpy as np
import ml_dtypes

import concourse.bass as bass
import concourse.mybir as mybir
import concourse.tile as tile
from concourse import bacc
from concourse.bass import ts
from concourse.bass_utils import run_bass_kernel_spmd
from concourse.tile_rust import add_dep_helper

B, S, D = 2, 2048, 1024
H, DK = 16, 64
NCORE = 8
T = B * S          # 4096 tokens
TT = 512           # token tile (projections, q-tiles)
NT = T // TT       # 8
KC = 128           # key chunk
NJ = S // TT       # 4 q-tiles per batch
GG = 256           # a2a token group (per dst core, per batch)
SCALE = 1.0 / np.sqrt(DK)

f32 = mybir.dt.float32
bf16 = mybir.dt.bfloat16
EXP = mybir.ActivationFunctionType.Exp
MULT = mybir.AluOpType.mult
npbf = ml_dtypes.bfloat16


def build_program():
    nc = bacc.Bacc("TRN2", target_bir_lowering=False, debug=False,
                   num_devices=NCORE)

    xT_d = nc.dram_tensor("xT", [NT, 128, 8, TT], bf16, kind="ExternalInput").ap()
    wT_d = nc.dram_tensor("wT", [128, 8, 3, 128], bf16, kind="ExternalInput").ap()
    woT_d = nc.dram_tensor("woT", [128, 8, 1024], bf16, kind="ExternalInput").ap()
    bqkv_d = nc.dram_tensor("bqkv", [128, 3], f32, kind="ExternalInput").ap()
    bo_d = nc.dram_tensor("bo", [128, 1024], f32, kind="ExternalInput").ap()
    trimask_d = nc.dram_tensor("trimask", [128, 128], bf16, kind="ExternalInput").ap()
    ident_d = nc.dram_tensor("ident", [128, 128], bf16, kind="ExternalInput").ap()
    outT_d = nc.dram_tensor("outT", [B, 2, 128, 1024], f32, kind="ExternalOutput").ap()

    with tile.TileContext(nc) as tc:
        with (
            tc.tile_pool(name="const", bufs=1) as constp,
            tc.tile_pool(name="wostream", bufs=1) as wop,
            tc.tile_pool(name="xstream", bufs=2) as xp,
            tc.tile_pool(name="qkv", bufs=NT) as qkvp,
            tc.tile_pool(name="vaug", bufs=NJ) as vaugp,
            tc.tile_pool(name="ptile", bufs=4) as pp,
            tc.tile_pool(name="post", bufs=2) as postp,
            tc.tile_pool(name="cxn", bufs=4) as cxnp,
            tc.tile_pool(name="cxhold", bufs=4) as cxp,
            tc.tile_pool(name="outsb", bufs=2) as outp,
            tc.tile_pool(name="ps_s", bufs=2, space="PSUM") as ps_s,
            tc.tile_pool(name="ps_ctx", bufs=1, space="PSUM") as ps_ctx,
            tc.tile_pool(name="ps_misc", bufs=2, space="PSUM") as ps_misc,
            tc.tile_pool(name="dram", bufs=1, space="DRAM") as dramp,
        ):
            # ---- constants on the Scalar HWDGE ring (parallel with x) ----
            wT = constp.tile([128, 8, 3, 128], bf16, tag="wT")
            nc.scalar.dma_start(wT[:], wT_d)
            bqkv = constp.tile([128, 3], f32, tag="bqkv")
            nc.scalar.dma_start(bqkv[:], bqkv_d)
            ident = constp.tile([128, 128], bf16, tag="ident")
            nc.scalar.dma_start(ident[:], ident_d)
            trimask = constp.tile([128, 128], bf16, tag="trimask")
            nc.scalar.dma_start(trimask[:], trimask_d)

            # per-token-tile Q/K/V (transposed) and per-tile augmented V
            qkv_t = [[None] * NT for _ in range(3)]   # [j][t] -> [128, TT]
            vaug_t = [[[None] * NJ for _ in range(2)] for _ in range(B)]

            # one A2A per batch: dst core c <- tokens [256c, +256) of each batch
            a2a_in = {b: dramp.tile([NCORE, 128, GG], bf16, name=f"a2a_in{b}")
                      for b in range(B)}
            a2a_out = {b: dramp.tile([NCORE, 128, GG], bf16, name=f"a2a_out{b}")
                       for b in range(B)}
            warm_in = dramp.tile([NCORE, 4, 32], f32, name="warm_in")
            warm_out = dramp.tile([NCORE, 4, 32], f32, name="warm_out")

            last_chain_dma = [None]

            def proj_tile(t):
                xt = xp.tile([128, 8, TT], bf16, tag="xt")
                if t == 0:
                    # split the first tile so the opening matmul starts after
                    # ~256KB instead of the full megabyte
                    nc.sync.dma_start(xt[:, 0:2, :], xT_d[t, :, 0:2, :])
                    nc.sync.dma_start(xt[:, 2:8, :], xT_d[t, :, 2:8, :])
                else:
                    nc.sync.dma_start(xt[:], xT_d[t])
                for j in range(3):
                    ps = ps_misc.tile([128, TT], f32, tag="mm")
                    for o in range(8):
                        nc.tensor.matmul(ps[:], wT[:, o, j, :], xt[:, o, :],
                                         start=(o == 0), stop=(o == 7))
                    qt = qkvp.tile([128, TT], bf16, tag=f"qkv{j}",
                                   name=f"qkv{j}_{t}")
                    nc.vector.tensor_scalar_add(qt[:], ps[:], bqkv[:, j:j + 1])
                    qkv_t[j][t] = qt

            def vtrans_tile(t):
                b, tl = t // NJ, t % NJ
                va = [vaugp.tile([128, NJ, DK + 1], bf16, tag=f"va{b}{h}",
                                 name=f"va{b}{h}_{tl}") for h in range(2)]
                for h in range(2):
                    nc.vector.memset(va[h][:, :, DK:DK + 1], 1.0)
                    vaug_t[b][h][tl] = va[h]
                for kt in range(NJ):
                    ps_t = ps_misc.tile([128, TT], bf16, tag="mm")
                    nc.tensor.transpose(ps_t[:, 0:128],
                                        qkv_t[2][t][:, kt * KC:(kt + 1) * KC],
                                        ident[:])
                    for h in range(2):
                        nc.vector.tensor_copy(va[h][:, kt, 0:DK],
                                              ps_t[:, DK * h:DK * h + DK])

            def attention_qtile(b, j):
                nk = 4 * (j + 1)
                pc = [ps_ctx.tile([DK + 1, TT], f32, tag=f"c{h}", name=f"pc{h}")
                      for h in range(2)]

                def emit_pv(p_tile, m):
                    for h in range(2):
                        nc.tensor.matmul(
                            pc[h][:], vaug_t[b][h][m // 4][:, m % 4, :],
                            p_tile[:, TT * h:TT * (h + 1)],
                            start=(m == 0), stop=(m == nk - 1),
                            skip_group_check=True)

                qt = qkv_t[0][b * NJ + j]
                pending = []
                for m in range(nk):
                    kt_tile = qkv_t[1][b * NJ + m // 4]
                    ko = (m % 4) * KC
                    ps = ps_s.tile([128, 2 * TT], f32, tag="s")
                    nc.tensor.matmul(ps[:, 0:TT], kt_tile[0:DK, ko:ko + KC],
                                     qt[0:DK, :],
                                     start=True, stop=True, tile_position=(0, 0))
                    nc.tensor.matmul(ps[:, TT:], kt_tile[DK:128, ko:ko + KC],
                                     qt[DK:128, :],
                                     start=True, stop=True, tile_position=(64, 0))
                    p = pp.tile([128, 2 * TT], bf16, tag="p")
                    r = m - 4 * j
                    if r >= 0:
                        if r > 0:
                            nc.vector.memset(
                                p[:].rearrange("k (h q) -> k h q", h=2)[:, :, 0:KC * r],
                                0.0)
                        nc.scalar.activation(
                            p[:].rearrange("k (h q) -> k h q", h=2)[:, :, KC * r:],
                            ps[:].rearrange("k (h q) -> k h q", h=2)[:, :, KC * r:],
                            EXP, scale=float(SCALE))
                        nc.vector.tensor_tensor(
                            p[:].rearrange("k (h q) -> k h q", h=2)[:, :, KC * r:KC * (r + 1)],
                            p[:].rearrange("k (h q) -> k h q", h=2)[:, :, KC * r:KC * (r + 1)],
                            trimask[:, None, :].to_broadcast([128, 2, 128]), MULT)
                    else:
                        nc.scalar.activation(p[:], ps[:], EXP, scale=float(SCALE))
                    pending.append((p, m))
                    if len(pending) > 2:   # depth-2: PE never waits on a fresh exp
                        emit_pv(*pending.pop(0))
                for pm in pending:
                    emit_pv(*pm)

                # per-q-tile softmax normalization + ship to the A2A buffer.
                # cx/rtmp copies come first so the ctx PSUM banks free up
                # before the DVE queue hits the DMA-gated reciprocal; the tiny
                # gather DMAs ride the idle GpSimd SWDGE channel instead of
                # queueing behind megabyte x-tile loads on Sync.
                rs_g = postp.tile([128, 8], f32, tag="rsg")
                cxs = []
                for h in range(2):
                    rtmp = cxnp.tile([1, TT], f32, tag="rtmp")
                    nc.vector.tensor_copy(rtmp[:], pc[h][DK:DK + 1, :])
                    cx = cxp.tile([DK, TT], f32, tag="cx")
                    nc.vector.tensor_copy(cx[:], pc[h][0:DK, :])
                    cxs.append(cx)
                    nc.gpsimd.dma_start(rs_g[:, 4 * h:4 * h + 4], rtmp[:])
                rc_g = postp.tile([128, 8], f32, tag="rcg")
                with nc.allow_low_precision(reason="softmax denominator"):
                    nc.vector.reciprocal(rc_g[:], rs_g[:])
                for h in range(2):
                    cx = cxs[h]
                    rrow = cxnp.tile([1, TT], f32, tag="rrow")
                    nc.gpsimd.dma_start(rrow[:], rc_g[:, 4 * h:4 * h + 4])
                    bcast = cxnp.tile([DK, TT], f32, tag="bcast")
                    nc.gpsimd.partition_broadcast(bcast[:], rrow[:], channels=DK)
                    cxn = cxnp.tile([DK, TT], bf16, tag="cxn")
                    nc.vector.tensor_tensor(cxn[:], cx[:], bcast[:], MULT)
                    for g in range(2):   # 256-token groups -> dst cores 2j+g
                        dma = nc.sync.dma_start(
                            a2a_in[b][2 * j + g, DK * h:DK * (h + 1), :],
                            cxn[:, GG * g:GG * (g + 1)])
                        last_chain_dma[0] = dma

            def do_a2a(b):
                nc.gpsimd.collective_compute(
                    "AllToAll", mybir.AluOpType.bypass,
                    replica_groups=[list(range(NCORE))],
                    ins=[a2a_in[b][:].opt()], outs=[a2a_out[b][:].opt()])

            ctx_tiles = {}

            def load_ctx(b, anchor):
                ctx_sb = constp.tile([128, 8, GG], bf16, tag=f"ctx{b}",
                                     name=f"ctx{b}")
                # one DMA per source rank; gate behind the given chain
                # anchor so the scheduler can't hoist the collective wait
                # ahead of attention-critical DMAs on the same queue.
                for d in range(8):
                    dma = nc.sync.dma_start(ctx_sb[:, d, :], a2a_out[b][d])
                    if anchor is not None:
                        add_dep_helper(dma.ins, anchor.ins, sync=False,
                                       reason="don't hoist a2a-gated ctx DMA")
                ctx_tiles[b] = ctx_sb

            def outproj(b):
                ctx_sb = ctx_tiles[b]
                # natural orientation: out[tok, od] = ctx_chunk.T @ woT_chunk
                for th in range(2):          # 128-token halves
                    for oh in range(2):      # 512-wide od halves
                        ps = ps_misc.tile([128, TT], f32, tag="mm")
                        for d in range(8):
                            nc.tensor.matmul(
                                ps[:], ctx_sb[:, d, KC * th:KC * (th + 1)],
                                wo_sb[:, d, TT * oh:TT * (oh + 1)],
                                start=(d == 0), stop=(d == 7))
                        ot = outp.tile([128, TT], f32, tag="ot")
                        nc.vector.tensor_tensor(
                            ot[:], ps[:], bo_sb[:, TT * oh:TT * (oh + 1)],
                            mybir.AluOpType.add)
                        nc.sync.dma_start(
                            outT_d[b, th, :, TT * oh:TT * (oh + 1)], ot[:])

            # ---- fully pipelined schedule (projection one tile ahead) ----
            # warm up the CC core first: the first collective pays ~15us of
            # launch latency; burn it on a dummy while the PE does projections
            do_a2a_warm = nc.gpsimd.collective_compute(
                "AllToAll", mybir.AluOpType.bypass,
                replica_groups=[list(range(NCORE))],
                ins=[warm_in[:].opt()], outs=[warm_out[:].opt()])
            proj_tile(0)
            wo_sb = wop.tile([128, 8, 1024], bf16, tag="wo")
            nc.gpsimd.dma_start(wo_sb[:], woT_d)
            bo_sb = wop.tile([128, 1024], f32, tag="bobc")
            nc.gpsimd.dma_start(bo_sb[:], bo_d)
            for g in range(NT):
                if g + 1 < NT:
                    proj_tile(g + 1)
                vtrans_tile(g)
                attention_qtile(g // NJ, g % NJ)
                if g == NJ - 1:
                    do_a2a(0)
            do_a2a(1)
            # ctx DMAs are anchored on the last chain write so the scheduler
            # can't enqueue their collective waits ahead of attention DMAs.
            load_ctx(0, last_chain_dma[0])
            load_ctx(1, last_chain_dma[0])
            outproj(0)
            outproj(1)

    nc.compile()
    return nc


def make_in_maps(x, Wq, bq, Wk, bk, Wv, bv, Wo, bo):
    x = np.asarray(x, np.float32)
    xT = np.ascontiguousarray(x.reshape(T, D).T)                  # [D, T]
    # [NT, 128, 8, TT]: xT_t[t, p, o, q] = xT[o*128+p, t*TT+q]
    xT_t = np.ascontiguousarray(
        xT.reshape(8, 128, NT, TT).transpose(2, 1, 0, 3)).astype(npbf)

    woT = np.ascontiguousarray(
        np.asarray(Wo, np.float32).T.reshape(8, 128, 1024)
        .transpose(1, 0, 2)).astype(npbf)
    bo_bc = np.ascontiguousarray(
        np.broadcast_to(np.asarray(bo, np.float32)[None, :], (128, 1024)))

    trimask = (np.arange(128)[:, None] <= np.arange(128)[None, :]).astype(npbf)
    ident = np.eye(128, dtype=npbf)

    in_maps = []
    for c in range(NCORE):
        sl = slice(128 * c, 128 * (c + 1))
        wT_c = np.stack(
            [np.ascontiguousarray(
                np.asarray(W, np.float32)[sl, :].T.reshape(8, 128, 128)
                .transpose(1, 0, 2))
             for W in (Wq, Wk, Wv)], axis=2)                       # [128, 8, 3, 128]
        bqkv_c = np.stack([np.asarray(b_, np.float32)[sl]
                           for b_ in (bq, bk, bv)], axis=1)        # [128, 3]
        in_maps.append({
            "xT": xT_t,
            "wT": np.ascontiguousarray(wT_c).astype(npbf),
            "woT": woT,
            "bqkv": np.ascontiguousarray(bqkv_c),
            "bo": bo_bc,
            "trimask": trimask,
            "ident": ident,
        })
    return in_maps


def assemble_output(results):
    # results[c]["outT"]: [B, 2, 128, 1024] = out[(b, 256c + th*128 + t), od]
    out = np.empty((B, S, D), np.float32)
    for c in range(NCORE):
        out[:, GG * c:GG * (c + 1), :] = results[c]["outT"].reshape(B, GG, D)
    return out


_PROGRAM = None


def get_program():
    global _PROGRAM
    if _PROGRAM is None:
        _PROGRAM = build_program()
    return _PROGRAM


def run(in_maps, **kwargs):
    nc = get_program()
    return run_bass_kernel_spmd(nc, in_maps, core_ids=list(range(NCORE)), **kwargs)


def kernel(x, Wq, bq, Wk, bk, Wv, bv, Wo, bo):
    in_maps = make_in_maps(x, Wq, bq, Wk, bk, Wv, bv, Wo, bo)
    res = run(in_maps)
    return assemble_output(res.results)


if __name__ == "__main__":
    rng = np.random.default_rng(0)
    x = rng.standard_normal((B, S, D), dtype=np.float32)
    mk = lambda *s: ((rng.random(s).astype(np.float32)) - 0.5) / 16
    out = kernel(x, mk(D, D), mk(D), mk(D, D), mk(D), mk(D, D), mk(D),
                 mk(D, D), mk(D))
    print(out.shape, out.dtype, np.abs(out).mean())
